# revision 1
# baseline (speedup 1.0000x reference)
"""Softclamped multi-head attention (B=2, N=2048, DIM=1024, 16 heads x 64) on
8 TRN2 NeuronCores.

Sharding: tensor-parallel over heads — 2 heads per core. Each core computes its
heads' Q/K/V projections, attention, and a partial output projection; the 8
fp32 partials are summed on the host (the out-proj contraction dim is sharded),
so the device graph needs no collectives.

Device pipeline per core (all TensorE-facing data in bf16, accumulation fp32):
  1. DMA-transpose bf16 tokens [4096,1024] -> tokT [dim, tok] chunks.
  2. RMSNorm: sum-of-squares per token via ones-matmul column sums (gives the
     per-token scale replicated across partitions for free), rsqrt, apply.
  3. Projections produce qT/kT in [d, tok] layout directly and v in [tok, d].
  4. Q/K L2 head-norms via the same ones-matmul trick; (gamma+1) scales for q
     and k are folded into a single per-dim vector g2 applied on the q side.
  5. Per (batch, head): simT = kT^T@qT -> ACT tanh(x/50) -> ACT exp(6.25*x)
     (softclamp+scale constants ride ACT's free affine) -> PV matmul where v is
     augmented with 64 ones-columns so softmax denominators land in the psum
     partition-replicated; normalize; out-proj; DMA fp32 partial out.
"""

import os
os.environ.setdefault("JAX_PLATFORMS", "axon")
import sys
if "/opt/trn_rl_repo" not in sys.path:
    sys.path.insert(0, "/opt/trn_rl_repo")

import numpy as np
import ml_dtypes

import concourse.bass as bass  # noqa: F401
from concourse import bacc, mybir
import concourse.tile as tile
from concourse.bass_utils import run_bass_kernel_spmd

B, N, DIM = 2, 2048, 1024
H, DH = 16, 64
NCORES = 8
HPC = H // NCORES          # heads per core = 2
CD = HPC * DH              # per-core projection width = 128
T = B * N                  # 4096 tokens
DCH = DIM // 128           # 8 dim chunks
F32 = mybir.dt.float32
BF16 = mybir.dt.bfloat16
AF = mybir.ActivationFunctionType
MUL = mybir.AluOpType.mult

SOFTCLAMP = 50.0
SCALE = DH ** -0.5         # 1/8
RMS_EPS = 1e-6


def build_nc(debug_outs=False):
    nc = bacc.Bacc("TRN2", target_bir_lowering=False, debug=False,
                   num_devices=NCORES)
    tok = nc.declare_dram_parameter("tok", [DIM, T], BF16, isOutput=False)
    wq = nc.declare_dram_parameter("wq", [128, DCH * CD], BF16,
                                   isOutput=False)
    wk = nc.declare_dram_parameter("wk", [128, DCH * CD], BF16,
                                   isOutput=False)
    wv = nc.declare_dram_parameter("wv", [128, DCH * CD], BF16,
                                   isOutput=False)
    wo = nc.declare_dram_parameter("wo", [CD, DIM], BF16, isOutput=False)
    g2 = nc.declare_dram_parameter("g2", [CD, 1], F32, isOutput=False)
    out = nc.declare_dram_parameter("out", [T, DIM], BF16, isOutput=True)
    dbg = None
    if debug_outs:
        dbg = {
            "d_s": nc.declare_dram_parameter("d_s", [128, T], BF16, True),
            "d_qT": nc.declare_dram_parameter("d_qT", [128, T], BF16, True),
            "d_kT": nc.declare_dram_parameter("d_kT", [128, T], BF16, True),
            "d_v": nc.declare_dram_parameter("d_v", [128, T // 128, 256],
                                             BF16, True),
            "d_a0": nc.declare_dram_parameter("d_a0", [128, N], BF16, True),
            "d_a1": nc.declare_dram_parameter("d_a1", [128, N], BF16, True),
        }

    with tile.TileContext(nc) as tc:
        _emit(nc, tc, tok, wq, wk, wv, wo, g2, out, dbg)
    nc.compile()
    return nc


def _emit(nc, tc, tok, wq, wk, wv, wo, g2, out, dbg=None):
    from concourse.masks import make_identity

    with tc.tile_pool(name="const", bufs=1) as const, \
         tc.tile_pool(name="core", bufs=1) as core:

        # ---- constants / weights ----
        ones_bf = const.tile([128, 128], BF16, tag="ones")
        nc.vector.memset(ones_bf[:], 1.0)
        ident = const.tile([128, 128], BF16, tag="ident")
        make_identity(nc, ident[:])
        bias0 = const.tile([128, 1], F32, tag="bias0")
        nc.vector.memset(bias0[:], 0.0)
        bias_eps = const.tile([128, 1], F32, tag="bias_eps")
        nc.vector.memset(bias_eps[:], RMS_EPS)
        g2_sb = const.tile([128, 1], F32, tag="g2")
        nc.sync.dma_start(out=g2_sb[:], in_=g2[:])
        wq_sb = const.tile([128, DCH, CD], BF16, tag="wq")
        wk_sb = const.tile([128, DCH, CD], BF16, tag="wk")
        wv_sb = const.tile([128, DCH, CD], BF16, tag="wv")
        for w_dram, w_sb in ((wq, wq_sb), (wk, wk_sb), (wv, wv_sb)):
            nc.sync.dma_start(out=w_sb[:],
                              in_=w_dram.rearrange("p (c m) -> p c m", c=DCH))
        wo_sb = const.tile([128, DIM], BF16, tag="wo")
        nc.sync.dma_start(out=wo_sb[:], in_=wo[:])

        # persistent across phases A-D
        qT = core.tile([128, T], BF16, tag="qT")
        kT = core.tile([128, T], BF16, tag="kT")
        vT = core.tile([128, T], BF16, tag="vT")
        # v layout per 128-token chunk: [vA(64) | onesA(64) | onesB(64) | vB(64)]
        v_sb = core.tile([128, T // 128, 256], BF16, tag="v")
        nc.vector.memset(v_sb[:, :, 64:192], 1.0)
        s_bf = core.tile([128, T], BF16, tag="sbf")

        with tc.tile_pool(name="tokp", bufs=1) as tokp:
            with tc.tile_pool(name="psa", bufs=2, space="PSUM") as psa, \
                 tc.tile_pool(name="pa", bufs=2) as pa, \
                 tc.tile_pool(name="pa1", bufs=1) as pa1:
                # ---- phase A: transpose tokens; rms sum-of-squares ----
                tok_ch = [tokp.tile([128, T], BF16, tag=f"tok{ch}",
                                    name=f"tok{ch}") for ch in range(DCH)]
                ss0 = psa.tile([128, 2048], F32, tag="ps", name="ss0")
                ss1 = psa.tile([128, 2048], F32, tag="ps", name="ss1")
                for ch in range(DCH):
                    eng = nc.sync if ch % 2 == 0 else nc.scalar
                    eng.dma_start(
                        out=tok_ch[ch][:],
                        in_=tok[ch * 128:(ch + 1) * 128, :])
                    sq = pa.tile([128, T], BF16, tag="sq")
                    nc.vector.tensor_mul(sq[:], tok_ch[ch][:], tok_ch[ch][:])
                    for th in range(8):
                        sst = ss0 if th < 4 else ss1
                        nc.tensor.matmul(
                            sst[:, (th % 4) * 512:(th % 4 + 1) * 512],
                            ones_bf[:], sq[:, th * 512:(th + 1) * 512],
                            start=(ch == 0), stop=(ch == DCH - 1))
                # s = 1/sqrt(ss/DIM + eps), bf16, replicated on all partitions
                sA = pa1.tile([128, T], F32, tag="sA")
                nc.scalar.activation(sA[:, 0:2048], ss0[:], AF.Sqrt,
                                     bias=bias_eps[:], scale=1.0 / DIM)
                nc.scalar.activation(sA[:, 2048:4096], ss1[:], AF.Sqrt,
                                     bias=bias_eps[:], scale=1.0 / DIM)
                sB = pa1.tile([128, T], F32, tag="sB")
                nc.vector.reciprocal_approx_fast(sB[:], sA[:])
                nc.vector.tensor_copy(s_bf[:], sB[:])
                if dbg:
                    nc.sync.dma_start(out=dbg["d_s"][:], in_=s_bf[:])

            # ---- phase C: projections (RMS scale fused into epilogue),
            # with q/k norm chains interleaved into the next tensor's
            # projection stream so DVE/ACT chain latency hides under PE --
            with tc.tile_pool(name="psc", bufs=1, space="PSUM") as psc, \
                 tc.tile_pool(name="pc", bufs=2) as pc:
                if True:
                    def proj_group(w_sb, dstT, th):
                        tsl = slice(th * 512, (th + 1) * 512)
                        pq = psc.tile([128, 512], F32, tag="pq", name="pq",
                                      bufs=3)
                        for ch in range(DCH):
                            nc.tensor.matmul(
                                pq[:, 0:512], w_sb[:, ch, :],
                                tok_ch[ch][:, tsl],
                                start=(ch == 0), stop=(ch == DCH - 1))
                        nc.vector.tensor_mul(dstT[:, tsl], pq[:, 0:512],
                                             s_bf[:, tsl])

                    def norm_chain(dstT, tp, is_q):
                        tfull = slice(tp * 1024, (tp + 1) * 1024)
                        squ = pc.tile([128, 1024], BF16, tag="squ")
                        nc.vector.tensor_mul(squ[:], dstT[:, tfull],
                                             dstT[:, tfull])
                        n2 = psc.tile([128, 2048], F32, tag="n2",
                                      name="n2", bufs=1)
                        # layout: [A tokens 0:1024 | B tokens 1024:2048]
                        for ti in range(2):
                            nc.tensor.matmul(
                                n2[:, ti * 512:(ti + 1) * 512],
                                ones_bf[0:64, :],
                                squ[0:64, ti * 512:(ti + 1) * 512],
                                start=True, stop=True)
                            nc.tensor.matmul(
                                n2[:, 1024 + ti * 512:1024 + (ti + 1) * 512],
                                ones_bf[64:128, :],
                                squ[64:128, ti * 512:(ti + 1) * 512],
                                start=True, stop=True)
                        nrm = pc.tile([128, 2048], F32, tag="nrm")
                        nc.scalar.activation(nrm[:], n2[:], AF.Sqrt,
                                             bias=bias0[:])
                        rq = pc.tile([128, 2048], F32, tag="rq")
                        nc.vector.reciprocal_approx_fast(rq[:], nrm[:])
                        sc_a, sc_b = (g2_sb[0:64], g2_sb[64:128]) \
                            if is_q else (1.0, 1.0)
                        tfull = slice(tp * 1024, (tp + 1) * 1024)
                        nc.vector.scalar_tensor_tensor(
                            out=dstT[0:64, tfull], in0=dstT[0:64, tfull],
                            scalar=sc_a, in1=rq[0:64, 0:1024],
                            op0=MUL, op1=MUL)
                        nc.vector.scalar_tensor_tensor(
                            out=dstT[64:128, tfull], in0=dstT[64:128, tfull],
                            scalar=sc_b, in1=rq[64:128, 1024:2048],
                            op0=MUL, op1=MUL)

                    for th in range(8):
                        proj_group(wq_sb, qT, th)
                    for th in range(8):
                        proj_group(wk_sb, kT, th)
                        if th % 2 == 1:
                            norm_chain(qT, th // 2, True)
                    for th in range(8):
                        proj_group(wv_sb, vT, th)
                        if th % 2 == 1:
                            norm_chain(kT, th // 2, False)
                        # v transposes: vT [c, t] -> v_sb [t(chunk), c]
                        for tv in range(th * 4, th * 4 + 4):
                            ptr = psc.tile([128, 128], BF16, tag="ptr",
                                           name="ptr", bufs=1)
                            nc.tensor.transpose(
                                ptr[:, 0:128],
                                vT[:, tv * 128:(tv + 1) * 128], ident[:])
                            nc.vector.tensor_copy(v_sb[:, tv, 0:64],
                                                  ptr[:, 0:64])
                            nc.vector.tensor_copy(v_sb[:, tv, 192:256],
                                                  ptr[:, 64:128])
                    if dbg:
                        nc.sync.dma_start(out=dbg["d_qT"][:], in_=qT[:])
                        nc.sync.dma_start(out=dbg["d_kT"][:], in_=kT[:])
                        nc.sync.dma_start(out=dbg["d_v"][:], in_=v_sb[:])

        # ---- phase D: attention (both heads fused per psum tile) ----
        with tc.tile_pool(name="psd", bufs=1, space="PSUM") as psd, \
             tc.tile_pool(name="pd", bufs=2) as pd, \
             tc.tile_pool(name="pat", bufs=1) as pat:
            attT = [pat.tile([128, N], BF16, tag=f"attT{b}", name=f"attT{b}")
                    for b in range(B)]
            for b in range(B):
                boff = b * N
                for ihalf in range(2):
                    ioff = boff + ihalf * 1024
                    sim2 = psd.tile([128, 2048], F32, tag="sim2", name="sim2")
                    outpA = psd.tile([128, 1024], F32, tag="outA", name="outA")
                    outpB = psd.tile([128, 1024], F32, tag="outB", name="outB")
                    for jch in range(N // 128):
                        jsl = slice(boff + jch * 128, boff + (jch + 1) * 128)
                        for ic in range(2):
                            isl = slice(ioff + ic * 512, ioff + (ic + 1) * 512)
                            nc.tensor.matmul(
                                sim2[:, ic * 512:(ic + 1) * 512],
                                kT[0:64, jsl], qT[0:64, isl],
                                start=True, stop=True)
                            nc.tensor.matmul(
                                sim2[:, 1024 + ic * 512:1024 + (ic + 1) * 512],
                                kT[64:128, jsl], qT[64:128, isl],
                                start=True, stop=True)
                        th_t = pd.tile([128, 2048], F32, tag="tanh",
                                       bufs=3)
                        nc.scalar.activation(th_t[:], sim2[:], AF.Tanh,
                                             bias=bias0[:],
                                             scale=1.0 / SOFTCLAMP)
                        ex = pd.tile([128, 2048], BF16, tag="exp",
                                     bufs=3)
                        nc.scalar.activation(ex[:], th_t[:], AF.Exp,
                                             bias=bias0[:],
                                             scale=SOFTCLAMP * SCALE)
                        tvix = b * 16 + jch
                        last = (jch == N // 128 - 1)
                        for ic in range(2):
                            nc.tensor.matmul(
                                outpA[:, ic * 512:(ic + 1) * 512],
                                v_sb[:, tvix, 0:128],
                                ex[:, ic * 512:(ic + 1) * 512],
                                start=(jch == 0), stop=last)
                            nc.tensor.matmul(
                                outpB[:, ic * 512:(ic + 1) * 512],
                                v_sb[:, tvix, 128:256],
                                ex[:, 1024 + ic * 512:1024 + (ic + 1) * 512],
                                start=(jch == 0), stop=last)
                    # normalize: A v-dims rows 0:64 (sums 64:128); B swapped
                    asl = slice(ihalf * 1024, (ihalf + 1) * 1024)
                    for outp, hrows, srows in (
                            (outpA, slice(0, 64), slice(64, 128)),
                            (outpB, slice(64, 128), slice(0, 64))):
                        stmp = pd.tile([128, 1024], F32, tag="stmp")
                        nc.vector.tensor_copy(stmp[:], outp[:])
                        rsum = pd.tile([128, 1024], F32, tag="rsum")
                        nc.vector.reciprocal_approx_fast(rsum[:], stmp[:])
                        rs2 = pd.tile([128, 1024], F32, tag="rs2")
                        nc.gpsimd.dma_start(out=rs2[hrows, :],
                                            in_=rsum[srows, :])
                        nc.vector.tensor_mul(attT[b][hrows, asl],
                                             outp[hrows, :], rs2[hrows, :])
                if dbg:
                    nc.sync.dma_start(out=dbg[f"d_a{b}"][:], in_=attT[b][:])

            # out projections for both batches, emitted after all attention
            # so they form one dense PE stream at the tail; stage 8
            # token-chunks per output DMA (2 MB each)
            for ob in range(B):
                for tg in range(4):
                    o_big = pd.tile([128, 4, DIM], BF16, tag="obig")
                    for ti in range(4):
                        tc_i = tg * 4 + ti
                        po = psd.tile([128, 1024], F32,
                                      tag=("outA", "outB", "sim2")[tc_i % 3],
                                      name="po")
                        for ec in range(2):
                            nc.tensor.matmul(
                                po[:, ec * 512:(ec + 1) * 512],
                                attT[ob][:, tc_i * 128:(tc_i + 1) * 128],
                                wo_sb[:, ec * 512:(ec + 1) * 512],
                                start=True, stop=True)
                        nc.scalar.activation(o_big[:, ti, 0:512],
                                             po[:, 0:512], AF.Copy)
                        nc.vector.tensor_copy(o_big[:, ti, 512:1024],
                                              po[:, 512:1024])
                    r0 = ob * N + tg * 512
                    nc.sync.dma_start(
                        out=out[r0:r0 + 512, :].rearrange(
                            "(t p) e -> p t e", p=128),
                        in_=o_big[:])


_NC = None


def _get_nc():
    global _NC
    if _NC is None:
        _NC = build_nc()
    return _NC


def _ensure_axon_hooks():
    """Install a fallback antenv.axon_hooks if the image lacks it, so
    trace=True degrades (or works via the boot ctypes hook) instead of
    crashing on import."""
    try:
        import antenv.axon_hooks  # noqa: F401
        return
    except ImportError:
        pass
    import types
    hook = None
    try:
        if "/root/.axon_site" not in sys.path:
            sys.path.insert(0, "/root/.axon_site")
        from trn_agent_boot.trn_boot import _ntff_profile_via_ctypes
        hook = _ntff_profile_via_ctypes("/opt/axon/libaxon_pjrt.so")
    except Exception:
        hook = None
    m = types.ModuleType("antenv.axon_hooks")
    m.get_axon_ntff_profile_hook = lambda: hook
    sys.modules["antenv.axon_hooks"] = m


def kernel(tokens, norm_w, w_q, w_kv, w_out, q_gamma, k_gamma):
    tokens = np.asarray(tokens, np.float32)
    norm_w = np.asarray(norm_w, np.float32)
    w_q = np.asarray(w_q, np.float32)
    w_kv = np.asarray(w_kv, np.float32)
    w_out = np.asarray(w_out, np.float32)
    q_gamma = np.asarray(q_gamma, np.float32)
    k_gamma = np.asarray(k_gamma, np.float32)

    bf = ml_dtypes.bfloat16
    wq_f = norm_w[:, None] * w_q
    wkv_f = norm_w[:, None] * w_kv
    wk_f = wkv_f[:, :H * DH]
    wv_f = wkv_f[:, H * DH:]
    tok_bf = np.ascontiguousarray(
        tokens.reshape(T, DIM).astype(bf).T)
    # combined q*k gamma scale (incl. both sqrt(DH) factors), applied on q side
    g2_full = ((q_gamma + 1.0) * (k_gamma + 1.0) * float(DH)).reshape(H * DH)

    def _swz(w):
        # [DIM, CD] -> device lhsT layout [p=128, (c, m)] contiguous
        return np.ascontiguousarray(
            w.astype(bf).reshape(DCH, 128, CD).transpose(1, 0, 2)
            .reshape(128, DCH * CD))

    in_maps = []
    for c in range(NCORES):
        cols = slice(c * CD, (c + 1) * CD)
        in_maps.append({
            "tok": tok_bf,
            "wq": _swz(wq_f[:, cols]),
            "wk": _swz(wk_f[:, cols]),
            "wv": _swz(wv_f[:, cols]),
            "wo": np.ascontiguousarray(w_out[cols, :]).astype(bf),
            "g2": np.ascontiguousarray(
                g2_full[c * CD:(c + 1) * CD].reshape(CD, 1), dtype=np.float32),
        })

    nc = _get_nc()
    trace = os.environ.get("KBENCH_TRACE") == "1"
    kwargs = {}
    if trace:
        _ensure_axon_hooks()
        import concourse.bass_utils as _bu
        _bu.upload_artifacts = lambda d: "local://" + d
        kwargs = {"trace": True,
                  "tmpdir": os.environ.get("KBENCH_TRACE_DIR") or None}
    res = run_bass_kernel_spmd(nc, in_maps, core_ids=list(range(NCORES)),
                               **kwargs)
    if res.exec_time_ns is not None:
        print(f"HW exec time: {res.exec_time_ns} ns")
    acc = np.zeros((T, DIM), np.float32)
    for i in range(NCORES):
        acc += res.results[i]["out"].astype(np.float32)
    return acc.reshape(B, N, DIM)


if __name__ == "__main__":
    rng = np.random.default_rng(0)
    inputs = {
        "tokens": rng.standard_normal((B, N, DIM), dtype=np.float32),
        "norm_w": np.ones((DIM,), np.float32),
        "w_q": rng.standard_normal((DIM, H * DH), dtype=np.float32) * 0.02,
        "w_kv": rng.standard_normal((DIM, 2 * H * DH), dtype=np.float32) * 0.02,
        "w_out": rng.standard_normal((H * DH, DIM), dtype=np.float32) * 0.02,
        "q_gamma": np.zeros((H, DH), np.float32),
        "k_gamma": np.zeros((H, DH), np.float32),
    }
    out = kernel(**inputs)
    print("out", out.shape, out.dtype, float(np.abs(out).max()))



# revision 11
# speedup vs baseline: 1.6236x; 1.6236x over previous
"""Softclamped multi-head attention (B=2, N=2048, DIM=1024, 16 heads x 64) on
8 TRN2 NeuronCores.

Sharding: tensor-parallel over heads - 2 heads per core. Each core computes its
heads' Q/K/V projections, attention, and a partial output projection; the 8
fp32 partials are summed on the host (the out-proj contraction dim is sharded),
so the device graph needs no collectives.

Engine split (the softmax nonlinearity dominates):
  - tanh(sim/50) is a degree-7 odd polynomial evaluated by a CUSTOM DVE op
    (one pass, PSUM fp32 in -> SBUF fp32 logits out), taking ~150us off the
    scalar (ACT) engine, which keeps only the exp pass.
  - RMS scale s cancels exactly in the q/k L2 head-norms, so only the V
    projection applies it; q/k projections don't wait on it.
  - q/k head-norms are computed partition-aligned (quadrant ones-matmuls) so
    the application is a single full-width tensor_tensor multiply, and the
    (gamma+1) scales ride the ACT rsqrt's per-partition scale vector.
  - Attention inner loop is software-pipelined: sim[j+1] matmuls are emitted
    before pv[j], with ping-ponged sim PSUM tiles, so the PE never sits behind
    the tanh/exp chain. PSUM budget: outpA+outpB (2 banks) + sim x2 (4) +
    out-proj po (2) = 8.
  - Softmax denominators ride the PV matmul as ones-columns of the augmented
    v tile [onesA | vA | vB | onesB]; normalization is two aligned TT mults
    after a gpsimd partition-shift of the reciprocals.
"""

import os
os.environ.setdefault("JAX_PLATFORMS", "axon")
import sys
if "/opt/trn_rl_repo" not in sys.path:
    sys.path.insert(0, "/opt/trn_rl_repo")

import numpy as np
import ml_dtypes

import concourse.bass as bass  # noqa: F401
from concourse import bacc, mybir
import concourse.tile as tile
from concourse.bass_utils import run_bass_kernel_spmd

B, N, DIM = 2, 2048, 1024
H, DH = 16, 64
NCORES = 8
HPC = H // NCORES          # heads per core = 2
CD = HPC * DH              # per-core projection width = 128
T = B * N                  # 4096 tokens
DCH = DIM // 128           # 8 dim chunks
F32 = mybir.dt.float32
F16 = mybir.dt.float16
BF16 = mybir.dt.bfloat16
U32 = mybir.dt.uint32
AF = mybir.ActivationFunctionType
MUL = mybir.AluOpType.mult
IB = 512                   # attention i-block (queries per phase-D block)
NBLK = T // IB             # 8 blocks
NJ = N // 128              # 16 key chunks per batch

SOFTCLAMP = 50.0
SCALE = DH ** -0.5         # 1/8
RMS_EPS = 1e-6
N_ACT_TANH = 1             # per block, # of trailing jch whose tanh runs on ACT

# ---------------------------------------------------------------------------
# Custom DVE op: logits = 6.25 * tanh_poly7(sim), evaluated directly on the
# raw sim value (the /50 is folded into the coefficients).
#   t(s) = s * (((D*u + C)*u + B)*u + A), u = s*s
# Minimax fit of tanh(s/50) on |s| <= 68; max abs err 4.6e-4 -> logit err
# <= 0.002 (weight rel err ~0.2%).
# ---------------------------------------------------------------------------
_PF = 6.25  # SOFTCLAMP * SCALE
TP_A = _PF * 0.99819183 / 50.0
TP_B = _PF * -0.31795733 / 50.0 ** 3
TP_C = _PF * 0.09607557 / 50.0 ** 5
TP_D = _PF * -0.01496778 / 50.0 ** 7

# normalized variant: inner poly has unit linear coefficient; the overall
# factor TP_A rides the ACT exp scale.  body uses only Src0 + 3 scalar
# slots + the hardware One constant (same input shape as the HW-validated
# RECIPROCAL_APPROX_FAST).
TP_BP = TP_B / TP_A
TP_CP = TP_C / TP_A
TP_DP = TP_D / TP_A

_TANH_OP = None


def _register_tanh_op():
    global _TANH_OP
    if _TANH_OP is not None:
        return _TANH_OP
    import concourse.dve_ops as dve_ops
    from concourse.dve_spec import (Spec, Src0, One, C0, C1, C2, lower,
                                    _has_src1)
    from concourse.dve_uop import DveOpSpec

    name = "TANH_POLY7B_ANT"
    if name in dve_ops._SUB_OPCODE_FOR_NAME:
        _TANH_OP = next(o for o in dve_ops.OPS if o.name == name)
        return _TANH_OP

    u = Src0 * Src0
    body = ((((u * C0) + C1) * u + C2) * u + One) * Src0

    def ref(in0, in1, s0, s1, imm2):
        x = in0.astype(np.float32)
        uu = x * x
        return ((((uu * s0) + s1) * uu + imm2) * uu + 1.0) * x

    spec = Spec(body=body, reference=ref)
    row = dve_ops._CUSTOM_DVE_ROW_BASE + len(dve_ops.OPS)
    assert row < 0x20
    dve_ops._SUB_OPCODE_FOR_NAME[name] = row
    shas = {}
    for ver in ("v3", "v4"):
        uops = lower(spec, ver=ver)
        shas[ver] = DveOpSpec(
            name=name, opcode=row, uops=uops, rd1_en=_has_src1(spec)
        ).sha(ver)
    op = dve_ops.DveOp(name, spec, subdim=False, uops_sha=shas)
    dve_ops.OPS.append(op)
    dve_ops.CUSTOM_DVE_SPECS[name] = op.spec
    _TANH_OP = op
    return op


def build_nc(debug_outs=False):
    op = _register_tanh_op()
    nc = bacc.Bacc("TRN2", target_bir_lowering=False, debug=False,
                   num_devices=NCORES)
    tok = nc.declare_dram_parameter("tok", [DIM, T], BF16, isOutput=False)
    wq = nc.declare_dram_parameter("wq", [128, DCH * CD], BF16, isOutput=False)
    wk = nc.declare_dram_parameter("wk", [128, DCH * CD], BF16, isOutput=False)
    wv = nc.declare_dram_parameter("wv", [128, DCH * CD], BF16, isOutput=False)
    wo = nc.declare_dram_parameter("wo", [CD, DIM], BF16, isOutput=False)
    # per-partition scale for the q-norm rsqrt: 1/g2^2 (g2 folds both gammas)
    gq = nc.declare_dram_parameter("gq", [CD, 1], F32, isOutput=False)
    out = nc.declare_dram_parameter("out", [T, DIM], BF16, isOutput=True)
    dbg = None
    if debug_outs:
        dbg = {
            "d_s": nc.declare_dram_parameter("d_s", [128, T], F32, True),
            "d_qT": nc.declare_dram_parameter("d_qT", [128, T], BF16, True),
            "d_kT": nc.declare_dram_parameter("d_kT", [128, T], BF16, True),
            "d_v": nc.declare_dram_parameter("d_v", [128, T // 128, 256],
                                             BF16, True),
            "d_att": nc.declare_dram_parameter("d_att", [128, T], BF16, True),
            "d_rs": nc.declare_dram_parameter("d_rs", [128, T], F32, True),
        }

    with tile.TileContext(nc) as tc:
        _emit(nc, tc, op, tok, wq, wk, wv, wo, gq, out, dbg)
    nc.compile()
    return nc


def _emit(nc, tc, tanh_op, tok, wq, wk, wv, wo, gq, out, dbg=None):
    from concourse.masks import make_identity

    with tc.tile_pool(name="const", bufs=1) as const, \
         tc.tile_pool(name="core", bufs=1) as core:

        # ---- constants / weights ----
        ones_bf = const.tile([128, 128], BF16, tag="ones")
        nc.vector.memset(ones_bf[:], 1.0)
        ident = const.tile([128, 128], BF16, tag="ident")
        make_identity(nc, ident[:])
        bias0 = const.tile([128, 1], F32, tag="bias0")
        nc.vector.memset(bias0[:], 0.0)
        bias_eps = const.tile([128, 1], F32, tag="bias_eps")
        nc.vector.memset(bias_eps[:], RMS_EPS)
        gq_sb = const.tile([128, 1], F32, tag="gq")
        nc.sync.dma_start(out=gq_sb[:], in_=gq[:])
        wq_sb = const.tile([128, DCH, CD], BF16, tag="wq")
        wk_sb = const.tile([128, DCH, CD], BF16, tag="wk")
        wv_sb = const.tile([128, DCH, CD], BF16, tag="wv")
        for w_dram, w_sb in ((wq, wq_sb), (wk, wk_sb), (wv, wv_sb)):
            nc.sync.dma_start(out=w_sb[:],
                              in_=w_dram.rearrange("p (c m) -> p c m", c=DCH))
        wo_sb = const.tile([128, DIM], BF16, tag="wo")
        nc.sync.dma_start(out=wo_sb[:], in_=wo[:])

        # persistent tensors
        qT = core.tile([128, T], BF16, tag="qT")
        kT = core.tile([128, T], BF16, tag="kT")
        vT = core.tile([128, T], BF16, tag="vT")
        # augmented v, per 128-token chunk: [onesA(64) | vA(64) | vB(64) | onesB(64)]
        v_sb = core.tile([128, T // 128, 256], BF16, tag="v")
        nc.vector.memset(v_sb[:, :, 0:64], 1.0)
        nc.vector.memset(v_sb[:, :, 192:256], 1.0)
        s_sb = core.tile([128, T], F16, tag="s")

        with tc.tile_pool(name="tokp", bufs=1) as tokp:
            tok_ch = [tokp.tile([128, T], BF16, tag=f"tok{ch}",
                                name=f"tok{ch}") for ch in range(DCH)]
            # ---- phase A: load tokens (transposed on host); rms sumsq ----
            with tc.tile_pool(name="psa", bufs=2, space="PSUM") as psa, \
                 tc.tile_pool(name="pa", bufs=2) as pa:
                ss0 = psa.tile([128, 2048], F32, tag="ps", name="ss0")
                ss1 = psa.tile([128, 2048], F32, tag="ps", name="ss1")
                for ch in range(DCH):
                    eng = nc.sync if ch % 2 == 0 else nc.scalar
                    eng.dma_start(out=tok_ch[ch][:],
                                  in_=tok[ch * 128:(ch + 1) * 128, :])
                    sq = pa.tile([128, T], BF16, tag="sq")
                    nc.vector.tensor_mul(sq[:], tok_ch[ch][:], tok_ch[ch][:])
                    for th in range(8):
                        sst = ss0 if th < 4 else ss1
                        nc.tensor.matmul(
                            sst[:, (th % 4) * 512:(th % 4 + 1) * 512],
                            ones_bf[:], sq[:, th * 512:(th + 1) * 512],
                            start=(ch == 0), stop=(ch == DCH - 1))
                # s = 1/sqrt(ss/DIM + eps), f16, replicated on all partitions
                nc.scalar.activation(s_sb[:, 0:2048], ss0[:],
                                     AF.Abs_reciprocal_sqrt,
                                     bias=bias_eps[:], scale=1.0 / DIM)
                nc.scalar.activation(s_sb[:, 2048:4096], ss1[:],
                                     AF.Abs_reciprocal_sqrt,
                                     bias=bias_eps[:], scale=1.0 / DIM)
                if dbg:
                    d = pa.tile([128, T], F32, tag="ds")
                    nc.vector.tensor_copy(d[:], s_sb[:])
                    nc.sync.dma_start(out=dbg["d_s"][:], in_=d[:])

            # ---- phase C: projections + head norms ----
            # q/k are stored unnormalized in bf16 (the rms scale cancels in
            # the L2 head-norm); v gets the rms scale in its epilogue.
            with tc.tile_pool(name="psc", bufs=1, space="PSUM") as psc, \
                 tc.tile_pool(name="pc", bufs=2) as pc:

                def proj_group(w_sb, dstT, th, with_s):
                    tsl = slice(th * 512, (th + 1) * 512)
                    pq = psc.tile([128, 512], F32, tag="pq", name="pq",
                                  bufs=3)
                    for ch in range(DCH):
                        nc.tensor.matmul(
                            pq[:, 0:512], w_sb[:, ch, :], tok_ch[ch][:, tsl],
                            start=(ch == 0), stop=(ch == DCH - 1))
                    if with_s:
                        nc.vector.tensor_mul(dstT[:, tsl], pq[:, 0:512],
                                             s_sb[:, tsl])
                    else:
                        nc.scalar.activation(dstT[:, tsl], pq[:, 0:512],
                                             AF.Copy)

                def norm_chain(dstT, tp, is_q):
                    # partition-aligned: n2[p, t] = sum of squares over the
                    # head that owns partition p (A: 0:64, B: 64:128)
                    tfull = slice(tp * 1024, (tp + 1) * 1024)
                    squ = pc.tile([128, 1024], BF16, tag="squ")
                    nc.vector.tensor_mul(squ[:], dstT[:, tfull],
                                         dstT[:, tfull])
                    n2 = psc.tile([128, 1024], F32, tag="n2", name="n2",
                                  bufs=1)
                    for ti in range(2):
                        csl = slice(ti * 512, (ti + 1) * 512)
                        nc.tensor.matmul(n2[0:64, csl], ones_bf[0:64, 0:64],
                                         squ[0:64, csl],
                                         start=True, stop=True)
                        nc.tensor.matmul(n2[64:128, csl],
                                         ones_bf[64:128, 0:64],
                                         squ[64:128, csl],
                                         start=True, stop=True)
                    # rq = g2/sqrt(n2) (q side) or 1/sqrt(n2) (k side)
                    rq = pc.tile([128, 1024], F16, tag="rq")
                    sc = gq_sb[:] if is_q else 1.0
                    nc.scalar.activation(rq[:], n2[:], AF.Abs_reciprocal_sqrt,
                                         bias=bias0[:], scale=sc)
                    nc.vector.tensor_mul(dstT[:, tfull], dstT[:, tfull],
                                         rq[:])

                for th in range(8):
                    proj_group(wq_sb, qT, th, False)
                for th in range(8):
                    proj_group(wk_sb, kT, th, False)
                    if th % 2 == 1:
                        norm_chain(qT, th // 2, True)
                for th in range(8):
                    proj_group(wv_sb, vT, th, True)
                    if th % 2 == 1:
                        norm_chain(kT, th // 2, False)
                    for tv in range(th * 4, th * 4 + 4):
                        ptr = psc.tile([128, 128], BF16, tag="ptr",
                                       name="ptr", bufs=2)
                        nc.tensor.transpose(
                            ptr[:, 0:128],
                            vT[:, tv * 128:(tv + 1) * 128], ident[:])
                        nc.vector.tensor_copy(
                            v_sb[:, tv, 64:192].bitcast(U32),
                            ptr[:, 0:128].bitcast(U32))
                if dbg:
                    nc.sync.dma_start(out=dbg["d_qT"][:], in_=qT[:])
                    nc.sync.dma_start(out=dbg["d_kT"][:], in_=kT[:])
                    nc.sync.dma_start(out=dbg["d_v"][:], in_=v_sb[:])

        # ---- phase D: attention, software-pipelined ----
        # Flat pipeline over NG = NBLK*NJ granules (block = IB queries,
        # granule = 128 keys x both heads). sim leads the tanh/exp chain by
        # 2 granules so the PE stays busy through the DVE->ACT latency.
        # PSUM: outpA+outpB (2 banks) + sim x2 (4) + po (2) = 8.
        with tc.tile_pool(name="psO", bufs=1, space="PSUM") as psO, \
             tc.tile_pool(name="psS", bufs=2, space="PSUM") as psS, \
             tc.tile_pool(name="psP", bufs=1, space="PSUM") as psP, \
             tc.tile_pool(name="pd", bufs=3) as pd, \
             tc.tile_pool(name="pe", bufs=2) as pe:

            NG = NBLK * NJ
            sims = [None] * NG
            exs = [None] * NG
            outp = {}
            pending = []  # deferred out-proj closures from finished blocks

            def ioff_of(blk):
                b, iq = blk // 4, blk % 4
                return b * N + iq * IB

            def emit_sim(g):
                blk, j = divmod(g, NJ)
                boff = (blk // 4) * N
                ioff = ioff_of(blk)
                jsl = slice(boff + j * 128, boff + (j + 1) * 128)
                isl = slice(ioff, ioff + IB)
                sim = psS.tile([128, 2 * IB], F32, tag="sim", name="sim")
                nc.tensor.matmul(sim[:, 0:IB], kT[0:64, jsl],
                                 qT[0:64, isl], start=True, stop=True)
                nc.tensor.matmul(sim[:, IB:2 * IB], kT[64:128, jsl],
                                 qT[64:128, isl], start=True, stop=True)
                sims[g] = sim

            def emit_act(g):
                sim = sims[g]
                ex = pd.tile([128, 2 * IB], BF16, tag="ex")
                lg = pd.tile([128, 2 * IB], F32, tag="lg")
                if g % NJ >= NJ - N_ACT_TANH:
                    # ACT-side tanh for engine load balance
                    nc.scalar.activation(lg[:], sim[:], AF.Tanh,
                                         bias=bias0[:], scale=1.0 / SOFTCLAMP)
                    nc.scalar.activation(ex[:], lg[:], AF.Exp,
                                         bias=bias0[:], scale=_PF)
                else:
                    nc.vector._custom_dve(
                        tanh_op, out=lg[:], in0=sim[:],
                        s0=TP_DP, s1=TP_CP, imm2=TP_BP)
                    nc.scalar.activation(ex[:], lg[:], AF.Exp,
                                         bias=bias0[:], scale=TP_A)
                exs[g] = ex

            def emit_pv(g):
                blk, j = divmod(g, NJ)
                if j == 0:
                    outp[blk] = (
                        psO.tile([128, IB], F32, tag="outA", name="outA"),
                        psO.tile([128, IB], F32, tag="outB", name="outB"))
                outpA, outpB = outp[blk]
                ex = exs[g]
                jv = (blk // 4) * NJ + j
                st = (j == 0)
                sp = (j == NJ - 1)
                nc.tensor.matmul(outpA[:, 0:IB], v_sb[:, jv, 0:128],
                                 ex[:, 0:IB], start=st, stop=sp)
                nc.tensor.matmul(outpB[:, 0:IB], v_sb[:, jv, 128:256],
                                 ex[:, IB:2 * IB], start=st, stop=sp)
                exs[g] = None

            def finish_block(blk):
                outpA, outpB = outp.pop(blk)
                # normalize: denA = outpA[0:64], vA-out = outpA[64:128];
                #            vB-out = outpB[0:64], denB = outpB[64:128]
                # custom-DVE ops misread partition-sliced PSUM APs, so the
                # reciprocals run on the full tiles (the non-denominator rows
                # produce unused garbage); cost is free-dim-bound anyway.
                ra = pe.tile([128, IB], F32, tag="ra")
                rb = pe.tile([128, IB], F32, tag="rb")
                nc.vector.reciprocal_approx_fast(ra[:], outpA[:])
                nc.vector.reciprocal_approx_fast(rb[:], outpB[:])
                rs = pe.tile([128, IB], F32, tag="rs")
                # partition shift: A-recips to 64:128, B-recips to 0:64
                nc.gpsimd.dma_start(out=rs[64:128, :], in_=ra[0:64, :])
                nc.gpsimd.dma_start(out=rs[0:64, :], in_=rb[64:128, :])
                # attT rows: [0:64] = head B dims, [64:128] = head A dims
                # (wo is host-reordered to match)
                attT = pe.tile([128, IB], BF16, tag="attT")
                nc.vector.tensor_mul(attT[0:64, :], outpB[0:64, :],
                                     rs[0:64, :])
                nc.vector.tensor_mul(attT[64:128, :], outpA[64:128, :],
                                     rs[64:128, :])
                if dbg:
                    io_ = ioff_of(blk)
                    nc.sync.dma_start(out=dbg["d_att"][:, io_:io_ + IB],
                                      in_=attT[:])
                    nc.sync.dma_start(out=dbg["d_rs"][:, io_:io_ + IB],
                                      in_=rs[:])

                # out-projection, deferred into the next block's loop
                o_big = pe.tile([128, IB // 128, DIM], BF16, tag="obig")
                ioff = ioff_of(blk)

                def mk(tci):
                    def go():
                        po = psP.tile([128, DIM], F32, tag="po", name="po")
                        for ec in range(2):
                            nc.tensor.matmul(
                                po[:, ec * 512:(ec + 1) * 512],
                                attT[:, tci * 128:(tci + 1) * 128],
                                wo_sb[:, ec * 512:(ec + 1) * 512],
                                start=True, stop=True)
                        nc.scalar.activation(o_big[:, tci, :], po[:], AF.Copy)
                        if tci == IB // 128 - 1:
                            nc.sync.dma_start(
                                out=out[ioff:ioff + IB, :].rearrange(
                                    "(t p) e -> p t e", p=128),
                                in_=o_big[:])
                    return go

                for tci in range(IB // 128):
                    pending.append(mk(tci))

            emit_sim(0)
            emit_sim(1)
            for g in range(NG):
                # act first: sim[g+2] reuses sim[g]'s PSUM buffer, so its
                # WAR dep needs tanh[g] emitted before the tile is recycled
                emit_act(g)
                if g + 2 < NG:
                    emit_sim(g + 2)
                if g % 4 == 2 and pending:
                    pending.pop(0)()
                emit_pv(g)
                if g % NJ == NJ - 1:
                    finish_block(g // NJ)
            for fn in pending:
                fn()


_NC = None


def _get_nc():
    global _NC
    if _NC is None:
        _NC = build_nc()
    return _NC


def _ensure_axon_hooks():
    """Install a fallback antenv.axon_hooks if the image lacks it, so
    trace=True degrades (or works via the boot ctypes hook) instead of
    crashing on import."""
    try:
        import antenv.axon_hooks  # noqa: F401
        return
    except ImportError:
        pass
    import types
    hook = None
    try:
        if "/root/.axon_site" not in sys.path:
            sys.path.insert(0, "/root/.axon_site")
        from trn_agent_boot.trn_boot import _ntff_profile_via_ctypes
        hook = _ntff_profile_via_ctypes("/opt/axon/libaxon_pjrt.so")
    except Exception:
        hook = None
    m = types.ModuleType("antenv.axon_hooks")
    m.get_axon_ntff_profile_hook = lambda: hook
    sys.modules["antenv.axon_hooks"] = m


def kernel(tokens, norm_w, w_q, w_kv, w_out, q_gamma, k_gamma):
    tokens = np.asarray(tokens, np.float32)
    norm_w = np.asarray(norm_w, np.float32)
    w_q = np.asarray(w_q, np.float32)
    w_kv = np.asarray(w_kv, np.float32)
    w_out = np.asarray(w_out, np.float32)
    q_gamma = np.asarray(q_gamma, np.float32)
    k_gamma = np.asarray(k_gamma, np.float32)

    bf = ml_dtypes.bfloat16
    wq_f = norm_w[:, None] * w_q
    wkv_f = norm_w[:, None] * w_kv
    wk_f = wkv_f[:, :H * DH]
    wv_f = wkv_f[:, H * DH:]
    tok_bf = np.ascontiguousarray(tokens.reshape(T, DIM).astype(bf).T)
    # combined q*k gamma scale (incl. both sqrt(DH) factors), applied on q side
    g2_full = ((q_gamma + 1.0) * (k_gamma + 1.0) * float(DH)).reshape(H * DH)

    def _swz(w):
        # [DIM, CD] -> device lhsT layout [p=128, (c, m)] contiguous
        return np.ascontiguousarray(
            w.astype(bf).reshape(DCH, 128, CD).transpose(1, 0, 2)
            .reshape(128, DCH * CD))

    in_maps = []
    for c in range(NCORES):
        cols = slice(c * CD, (c + 1) * CD)
        g2c = g2_full[c * CD:(c + 1) * CD]
        wo_c = w_out[cols, :]
        # attT rows are [head B dims, head A dims]
        wo_r = np.concatenate([wo_c[64:128, :], wo_c[0:64, :]], axis=0)
        in_maps.append({
            "tok": tok_bf,
            "wq": _swz(wq_f[:, cols]),
            "wk": _swz(wk_f[:, cols]),
            "wv": _swz(wv_f[:, cols]),
            "wo": np.ascontiguousarray(wo_r).astype(bf),
            "gq": np.ascontiguousarray(
                (1.0 / (g2c * g2c)).reshape(CD, 1), dtype=np.float32),
        })

    nc = _get_nc()
    trace = os.environ.get("KBENCH_TRACE") == "1"
    kwargs = {}
    if trace:
        _ensure_axon_hooks()
        import concourse.bass_utils as _bu
        _bu.upload_artifacts = lambda d: "local://" + d
        kwargs = {"trace": True,
                  "tmpdir": os.environ.get("KBENCH_TRACE_DIR") or None}
    res = run_bass_kernel_spmd(nc, in_maps, core_ids=list(range(NCORES)),
                               **kwargs)
    if res.exec_time_ns is not None:
        print(f"HW exec time: {res.exec_time_ns} ns")
    acc = np.zeros((T, DIM), np.float32)
    for i in range(NCORES):
        acc += res.results[i]["out"].astype(np.float32)
    return acc.reshape(B, N, DIM)


if __name__ == "__main__":
    rng = np.random.default_rng(0)
    inputs = {
        "tokens": rng.standard_normal((B, N, DIM), dtype=np.float32),
        "norm_w": np.ones((DIM,), np.float32),
        "w_q": rng.standard_normal((DIM, H * DH), dtype=np.float32) * 0.02,
        "w_kv": rng.standard_normal((DIM, 2 * H * DH), dtype=np.float32) * 0.02,
        "w_out": rng.standard_normal((H * DH, DIM), dtype=np.float32) * 0.02,
        "q_gamma": np.zeros((H, DH), np.float32),
        "k_gamma": np.zeros((H, DH), np.float32),
    }
    out = kernel(**inputs)
    print("out", out.shape, out.dtype, float(np.abs(out).max()))


# revision 14
# speedup vs baseline: 1.6549x; 1.0193x over previous
"""Softclamped multi-head attention (B=2, N=2048, DIM=1024, 16 heads x 64) on
8 TRN2 NeuronCores.

Sharding: tensor-parallel over heads - 2 heads per core. Each core computes its
heads' Q/K/V projections, attention, and a partial output projection; the 8
fp32 partials are summed on the host (the out-proj contraction dim is sharded),
so the device graph needs no collectives.

Engine split (the softmax nonlinearity dominates):
  - tanh(sim/50) is a degree-7 odd polynomial evaluated by a CUSTOM DVE op
    (one pass, PSUM fp32 in -> SBUF fp32 logits out), taking ~150us off the
    scalar (ACT) engine, which keeps only the exp pass.
  - RMS scale s cancels exactly in the q/k L2 head-norms, so only the V
    projection applies it; q/k projections don't wait on it.
  - q/k head-norms are computed partition-aligned (quadrant ones-matmuls) so
    the application is a single full-width tensor_tensor multiply, and the
    (gamma+1) scales ride the ACT rsqrt's per-partition scale vector.
  - Attention inner loop is software-pipelined: sim[j+1] matmuls are emitted
    before pv[j], with ping-ponged sim PSUM tiles, so the PE never sits behind
    the tanh/exp chain. PSUM budget: outpA+outpB (2 banks) + sim x2 (4) +
    out-proj po (2) = 8.
  - Softmax denominators ride the PV matmul as ones-columns of the augmented
    v tile [onesA | vA | vB | onesB]; normalization is two aligned TT mults
    after a gpsimd partition-shift of the reciprocals.
"""

import os
os.environ.setdefault("JAX_PLATFORMS", "axon")
import sys
if "/opt/trn_rl_repo" not in sys.path:
    sys.path.insert(0, "/opt/trn_rl_repo")

import numpy as np
import ml_dtypes

import concourse.bass as bass  # noqa: F401
from concourse import bacc, mybir
import concourse.tile as tile
from concourse.bass_utils import run_bass_kernel_spmd

B, N, DIM = 2, 2048, 1024
H, DH = 16, 64
NCORES = 8
HPC = H // NCORES          # heads per core = 2
CD = HPC * DH              # per-core projection width = 128
T = B * N                  # 4096 tokens
DCH = DIM // 128           # 8 dim chunks
F32 = mybir.dt.float32
F16 = mybir.dt.float16
BF16 = mybir.dt.bfloat16
U32 = mybir.dt.uint32
AF = mybir.ActivationFunctionType
MUL = mybir.AluOpType.mult
IB = 512                   # attention i-block (queries per phase-D block)
NBLK = T // IB             # 8 blocks
NJ = N // 128              # 16 key chunks per batch

SOFTCLAMP = 50.0
SCALE = DH ** -0.5         # 1/8
RMS_EPS = 1e-6
N_ACT_TANH = 0             # per block, # of trailing jch whose tanh runs on ACT

# ---------------------------------------------------------------------------
# Custom DVE op: logits = 6.25 * tanh_poly7(sim), evaluated directly on the
# raw sim value (the /50 is folded into the coefficients).
#   t(s) = s * (((D*u + C)*u + B)*u + A), u = s*s
# Minimax fit of tanh(s/50) on |s| <= 68; max abs err 4.6e-4 -> logit err
# <= 0.002 (weight rel err ~0.2%).
# ---------------------------------------------------------------------------
_PF = 6.25  # SOFTCLAMP * SCALE
TP_A = _PF * 0.99819183 / 50.0
TP_B = _PF * -0.31795733 / 50.0 ** 3
TP_C = _PF * 0.09607557 / 50.0 ** 5
TP_D = _PF * -0.01496778 / 50.0 ** 7

# normalized variant: inner poly has unit linear coefficient; the overall
# factor TP_A rides the ACT exp scale.  body uses only Src0 + 3 scalar
# slots + the hardware One constant (same input shape as the HW-validated
# RECIPROCAL_APPROX_FAST).
TP_BP = TP_B / TP_A
TP_CP = TP_C / TP_A
TP_DP = TP_D / TP_A

_TANH_OP = None


def _register_tanh_op():
    global _TANH_OP
    if _TANH_OP is not None:
        return _TANH_OP
    import concourse.dve_ops as dve_ops
    from concourse.dve_spec import (Spec, Src0, One, C0, C1, C2, lower,
                                    _has_src1)
    from concourse.dve_uop import DveOpSpec

    name = "TANH_POLY7B_ANT"
    if name in dve_ops._SUB_OPCODE_FOR_NAME:
        _TANH_OP = next(o for o in dve_ops.OPS if o.name == name)
        return _TANH_OP

    u = Src0 * Src0
    body = ((((u * C0) + C1) * u + C2) * u + One) * Src0

    def ref(in0, in1, s0, s1, imm2):
        x = in0.astype(np.float32)
        uu = x * x
        return ((((uu * s0) + s1) * uu + imm2) * uu + 1.0) * x

    spec = Spec(body=body, reference=ref)
    row = dve_ops._CUSTOM_DVE_ROW_BASE + len(dve_ops.OPS)
    assert row < 0x20
    dve_ops._SUB_OPCODE_FOR_NAME[name] = row
    shas = {}
    for ver in ("v3", "v4"):
        uops = lower(spec, ver=ver)
        shas[ver] = DveOpSpec(
            name=name, opcode=row, uops=uops, rd1_en=_has_src1(spec)
        ).sha(ver)
    op = dve_ops.DveOp(name, spec, subdim=False, uops_sha=shas)
    dve_ops.OPS.append(op)
    dve_ops.CUSTOM_DVE_SPECS[name] = op.spec
    _TANH_OP = op
    return op


def build_nc(debug_outs=False):
    op = _register_tanh_op()
    nc = bacc.Bacc("TRN2", target_bir_lowering=False, debug=False,
                   num_devices=NCORES)
    tok = nc.declare_dram_parameter("tok", [DIM, T], BF16, isOutput=False)
    wq = nc.declare_dram_parameter("wq", [128, DCH * CD], BF16, isOutput=False)
    wk = nc.declare_dram_parameter("wk", [128, DCH * CD], BF16, isOutput=False)
    wv = nc.declare_dram_parameter("wv", [128, DCH * CD], BF16, isOutput=False)
    wo = nc.declare_dram_parameter("wo", [CD, DIM], BF16, isOutput=False)
    # per-partition scale for the q-norm rsqrt: 1/g2^2 (g2 folds both gammas)
    gq = nc.declare_dram_parameter("gq", [CD, 1], F32, isOutput=False)
    out = nc.declare_dram_parameter("out", [T, DIM], BF16, isOutput=True)
    dbg = None
    if debug_outs:
        dbg = {
            "d_s": nc.declare_dram_parameter("d_s", [128, T], F32, True),
            "d_qT": nc.declare_dram_parameter("d_qT", [128, T], BF16, True),
            "d_kT": nc.declare_dram_parameter("d_kT", [128, T], BF16, True),
            "d_v": nc.declare_dram_parameter("d_v", [128, T // 128, 256],
                                             BF16, True),
            "d_att": nc.declare_dram_parameter("d_att", [128, T], BF16, True),
            "d_rs": nc.declare_dram_parameter("d_rs", [128, T], F32, True),
        }

    with tile.TileContext(nc) as tc:
        _emit(nc, tc, op, tok, wq, wk, wv, wo, gq, out, dbg)
    nc.compile()
    return nc


def _emit(nc, tc, tanh_op, tok, wq, wk, wv, wo, gq, out, dbg=None):
    from concourse.masks import make_identity

    with tc.tile_pool(name="const", bufs=1) as const, \
         tc.tile_pool(name="core", bufs=1) as core:

        # ---- constants / weights ----
        ones_bf = const.tile([128, 128], BF16, tag="ones")
        nc.vector.memset(ones_bf[:], 1.0)
        ident = const.tile([128, 128], BF16, tag="ident")
        make_identity(nc, ident[:])
        bias0 = const.tile([128, 1], F32, tag="bias0")
        nc.vector.memset(bias0[:], 0.0)
        bias_eps = const.tile([128, 1], F32, tag="bias_eps")
        nc.vector.memset(bias_eps[:], RMS_EPS)
        gq_sb = const.tile([128, 1], F32, tag="gq")
        nc.sync.dma_start(out=gq_sb[:], in_=gq[:])
        wq_sb = const.tile([128, DCH, CD], BF16, tag="wq")
        wk_sb = const.tile([128, DCH, CD], BF16, tag="wk")
        wv_sb = const.tile([128, DCH, CD], BF16, tag="wv")
        for w_dram, w_sb in ((wq, wq_sb), (wk, wk_sb), (wv, wv_sb)):
            nc.sync.dma_start(out=w_sb[:],
                              in_=w_dram.rearrange("p (c m) -> p c m", c=DCH))
        wo_sb = const.tile([128, DIM], BF16, tag="wo")
        nc.sync.dma_start(out=wo_sb[:], in_=wo[:])

        # persistent tensors
        qT = core.tile([128, T], BF16, tag="qT")
        kT = core.tile([128, T], BF16, tag="kT")
        vT = core.tile([128, T], BF16, tag="vT")
        # augmented v, per 128-token chunk: [onesA(64) | vA(64) | vB(64) | onesB(64)]
        v_sb = core.tile([128, T // 128, 256], BF16, tag="v")
        nc.vector.memset(v_sb[:, :, 0:64], 1.0)
        nc.vector.memset(v_sb[:, :, 192:256], 1.0)
        s_sb = core.tile([128, T], F16, tag="s")

        with tc.tile_pool(name="tokp", bufs=1) as tokp:
            tok_ch = [tokp.tile([128, T], BF16, tag=f"tok{ch}",
                                name=f"tok{ch}") for ch in range(DCH)]
            # ---- phase A: load tokens (transposed on host); rms sumsq ----
            with tc.tile_pool(name="psa", bufs=2, space="PSUM") as psa, \
                 tc.tile_pool(name="pa", bufs=2) as pa:
                ss0 = psa.tile([128, 2048], F32, tag="ps", name="ss0")
                ss1 = psa.tile([128, 2048], F32, tag="ps", name="ss1")
                for ch in range(DCH):
                    eng = nc.sync if ch % 2 == 0 else nc.scalar
                    eng.dma_start(out=tok_ch[ch][:],
                                  in_=tok[ch * 128:(ch + 1) * 128, :])
                    sq = pa.tile([128, T], BF16, tag="sq")
                    nc.vector.tensor_mul(sq[:], tok_ch[ch][:], tok_ch[ch][:])
                    for th in range(8):
                        sst = ss0 if th < 4 else ss1
                        nc.tensor.matmul(
                            sst[:, (th % 4) * 512:(th % 4 + 1) * 512],
                            ones_bf[:], sq[:, th * 512:(th + 1) * 512],
                            start=(ch == 0), stop=(ch == DCH - 1))
                # s = 1/sqrt(ss/DIM + eps), f16, replicated on all partitions
                nc.scalar.activation(s_sb[:, 0:2048], ss0[:],
                                     AF.Abs_reciprocal_sqrt,
                                     bias=bias_eps[:], scale=1.0 / DIM)
                nc.scalar.activation(s_sb[:, 2048:4096], ss1[:],
                                     AF.Abs_reciprocal_sqrt,
                                     bias=bias_eps[:], scale=1.0 / DIM)
                if dbg:
                    d = pa.tile([128, T], F32, tag="ds")
                    nc.vector.tensor_copy(d[:], s_sb[:])
                    nc.sync.dma_start(out=dbg["d_s"][:], in_=d[:])

            # ---- phase C: projections + head norms ----
            # q/k are stored unnormalized in bf16 (the rms scale cancels in
            # the L2 head-norm); v gets the rms scale in its epilogue.
            with tc.tile_pool(name="psc", bufs=1, space="PSUM") as psc, \
                 tc.tile_pool(name="pc", bufs=2) as pc:

                def proj_group(w_sb, dstT, th, with_s):
                    tsl = slice(th * 512, (th + 1) * 512)
                    pq = psc.tile([128, 512], F32, tag="pq", name="pq",
                                  bufs=3)
                    for ch in range(DCH):
                        nc.tensor.matmul(
                            pq[:, 0:512], w_sb[:, ch, :], tok_ch[ch][:, tsl],
                            start=(ch == 0), stop=(ch == DCH - 1))
                    if with_s:
                        nc.vector.tensor_mul(dstT[:, tsl], pq[:, 0:512],
                                             s_sb[:, tsl])
                    else:
                        nc.scalar.activation(dstT[:, tsl], pq[:, 0:512],
                                             AF.Copy)

                def norm_chain(dstT, tp, is_q):
                    # partition-aligned: n2[p, t] = sum of squares over the
                    # head that owns partition p (A: 0:64, B: 64:128)
                    tfull = slice(tp * 1024, (tp + 1) * 1024)
                    squ = pc.tile([128, 1024], BF16, tag="squ")
                    nc.vector.tensor_mul(squ[:], dstT[:, tfull],
                                         dstT[:, tfull])
                    n2 = psc.tile([128, 1024], F32, tag="n2", name="n2",
                                  bufs=1)
                    for ti in range(2):
                        csl = slice(ti * 512, (ti + 1) * 512)
                        nc.tensor.matmul(n2[0:64, csl], ones_bf[0:64, 0:64],
                                         squ[0:64, csl],
                                         start=True, stop=True)
                        nc.tensor.matmul(n2[64:128, csl],
                                         ones_bf[64:128, 0:64],
                                         squ[64:128, csl],
                                         start=True, stop=True)
                    # rq = g2/sqrt(n2) (q side) or 1/sqrt(n2) (k side)
                    rq = pc.tile([128, 1024], F16, tag="rq")
                    sc = gq_sb[:] if is_q else 1.0
                    nc.scalar.activation(rq[:], n2[:], AF.Abs_reciprocal_sqrt,
                                         bias=bias0[:], scale=sc)
                    nc.vector.tensor_mul(dstT[:, tfull], dstT[:, tfull],
                                         rq[:])

                for th in range(8):
                    proj_group(wq_sb, qT, th, False)
                    if th % 2 == 1:
                        norm_chain(qT, th // 2, True)
                for th in range(8):
                    proj_group(wk_sb, kT, th, False)
                    if th % 2 == 1:
                        norm_chain(kT, th // 2, False)
                for th in range(8):
                    proj_group(wv_sb, vT, th, True)
                    for tv in range(th * 4, th * 4 + 4):
                        ptr = psc.tile([128, 128], BF16, tag="ptr",
                                       name="ptr", bufs=2)
                        nc.tensor.transpose(
                            ptr[:, 0:128],
                            vT[:, tv * 128:(tv + 1) * 128], ident[:])
                        nc.vector.tensor_copy(
                            v_sb[:, tv, 64:192].bitcast(U32),
                            ptr[:, 0:128].bitcast(U32))
                if dbg:
                    nc.sync.dma_start(out=dbg["d_qT"][:], in_=qT[:])
                    nc.sync.dma_start(out=dbg["d_kT"][:], in_=kT[:])
                    nc.sync.dma_start(out=dbg["d_v"][:], in_=v_sb[:])

        # ---- phase D: attention, software-pipelined ----
        # Flat pipeline over NG = NBLK*NJ granules (block = IB queries,
        # granule = 128 keys x both heads). sim leads the tanh/exp chain by
        # 2 granules so the PE stays busy through the DVE->ACT latency.
        # PSUM: outpA+outpB (2 banks) + sim x2 (4) + po (2) = 8.
        with tc.tile_pool(name="psO", bufs=1, space="PSUM") as psO, \
             tc.tile_pool(name="psS", bufs=3, space="PSUM") as psS, \
             tc.tile_pool(name="pd", bufs=3) as pd, \
             tc.tile_pool(name="pe", bufs=2) as pe:

            NG = NBLK * NJ
            sims = [None] * NG
            exs = [None] * NG
            outp = {}
            pending = []  # deferred out-proj closures from finished blocks

            def ioff_of(blk):
                b, iq = blk // 4, blk % 4
                return b * N + iq * IB

            def emit_sim(g):
                blk, j = divmod(g, NJ)
                boff = (blk // 4) * N
                ioff = ioff_of(blk)
                jsl = slice(boff + j * 128, boff + (j + 1) * 128)
                isl = slice(ioff, ioff + IB)
                sim = psS.tile([128, 2 * IB], F32, tag="sim", name="sim")
                nc.tensor.matmul(sim[:, 0:IB], kT[0:64, jsl],
                                 qT[0:64, isl], start=True, stop=True)
                nc.tensor.matmul(sim[:, IB:2 * IB], kT[64:128, jsl],
                                 qT[64:128, isl], start=True, stop=True)
                sims[g] = sim

            # tanh granule: one [128, 2*IB] tile per g; exp fires once per
            # PAIR of granules on a [128, 4*IB] tile (halves ACT instr count)
            lgp = [None]
            exp_scale = [None]

            def emit_tanh(g):
                sim = sims[g]
                if g % 2 == 0:
                    lgp[0] = pd.tile([128, 4 * IB], F32, tag="lg", name="lgp")
                    exp_scale[0] = None
                lg = lgp[0][:, (g % 2) * 2 * IB:(g % 2 + 1) * 2 * IB]
                nc.vector._custom_dve(
                    tanh_op, out=lg, in0=sim[:],
                    s0=TP_DP, s1=TP_CP, imm2=TP_BP)

            def emit_exp_pair(g0):
                ex = pd.tile([128, 4 * IB], BF16, tag="ex")
                nc.scalar.activation(ex[:], lgp[0][:], AF.Exp,
                                     bias=bias0[:], scale=TP_A)
                exs[g0] = ex[:, 0:2 * IB]
                exs[g0 + 1] = ex[:, 2 * IB:4 * IB]

            def emit_pv(g):
                blk, j = divmod(g, NJ)
                if j == 0:
                    outp[blk] = (
                        psO.tile([128, IB], F32, tag="outA", name="outA"),
                        psO.tile([128, IB], F32, tag="outB", name="outB"))
                outpA, outpB = outp[blk]
                ex = exs[g]
                jv = (blk // 4) * NJ + j
                st = (j == 0)
                sp = (j == NJ - 1)
                nc.tensor.matmul(outpA[:, 0:IB], v_sb[:, jv, 0:128],
                                 ex[:, 0:IB], start=st, stop=sp)
                nc.tensor.matmul(outpB[:, 0:IB], v_sb[:, jv, 128:256],
                                 ex[:, IB:2 * IB], start=st, stop=sp)

            def finish_block(blk):
                outpA, outpB = outp.pop(blk)
                # normalize: denA = outpA[0:64], vA-out = outpA[64:128];
                #            vB-out = outpB[0:64], denB = outpB[64:128]
                # custom-DVE ops misread partition-sliced PSUM APs, so the
                # reciprocals run on the full tiles (the non-denominator rows
                # produce unused garbage); cost is free-dim-bound anyway.
                ra = pe.tile([128, IB], F32, tag="ra")
                rb = pe.tile([128, IB], F32, tag="rb")
                nc.vector.reciprocal_approx_fast(ra[:], outpA[:])
                nc.vector.reciprocal_approx_fast(rb[:], outpB[:])
                rs = pe.tile([128, IB], F32, tag="rs")
                # partition shift: A-recips to 64:128, B-recips to 0:64
                nc.gpsimd.dma_start(out=rs[64:128, :], in_=ra[0:64, :])
                nc.gpsimd.dma_start(out=rs[0:64, :], in_=rb[64:128, :])
                # attT rows: [0:64] = head B dims, [64:128] = head A dims
                # (wo is host-reordered to match)
                attT = pe.tile([128, IB], BF16, tag="attT")
                nc.vector.tensor_mul(attT[0:64, :], outpB[0:64, :],
                                     rs[0:64, :])
                nc.vector.tensor_mul(attT[64:128, :], outpA[64:128, :],
                                     rs[64:128, :])
                if dbg:
                    io_ = ioff_of(blk)
                    nc.sync.dma_start(out=dbg["d_att"][:, io_:io_ + IB],
                                      in_=attT[:])
                    nc.sync.dma_start(out=dbg["d_rs"][:, io_:io_ + IB],
                                      in_=rs[:])

                # out-projection, deferred into the next block's loop
                o_big = pe.tile([128, IB // 128, DIM], BF16, tag="obig")
                ioff = ioff_of(blk)

                def mk(tci):
                    def go():
                        # po rides the sim pool rotation (same shape/space)
                        po = psS.tile([128, DIM], F32, tag="sim", name="po")
                        for ec in range(2):
                            nc.tensor.matmul(
                                po[:, ec * 512:(ec + 1) * 512],
                                attT[:, tci * 128:(tci + 1) * 128],
                                wo_sb[:, ec * 512:(ec + 1) * 512],
                                start=True, stop=True)
                        nc.scalar.activation(o_big[:, tci, :], po[:], AF.Copy)
                        if tci == IB // 128 - 1:
                            nc.sync.dma_start(
                                out=out[ioff:ioff + IB, :].rearrange(
                                    "(t p) e -> p t e", p=128),
                                in_=o_big[:])
                    return go

                for tci in range(IB // 128):
                    pending.append(mk(tci))

            emit_sim(0)
            emit_sim(1)
            emit_sim(2)
            for g in range(NG):
                # tanh first: sim[g+3] reuses sim[g]'s PSUM buffer, so its
                # WAR dep needs tanh[g] emitted before the tile is recycled
                emit_tanh(g)
                if g % 2 == 1:
                    emit_exp_pair(g - 1)
                if g + 3 < NG:
                    emit_sim(g + 3)
                if g % 4 == 2 and pending:
                    pending.pop(0)()
                if g % 2 == 1:
                    emit_pv(g - 1)
                    emit_pv(g)
                if g % NJ == NJ - 1:
                    finish_block(g // NJ)
            for fn in pending:
                fn()


_NC = None


def _get_nc():
    global _NC
    if _NC is None:
        _NC = build_nc()
    return _NC


def _ensure_axon_hooks():
    """Install a fallback antenv.axon_hooks if the image lacks it, so
    trace=True degrades (or works via the boot ctypes hook) instead of
    crashing on import."""
    try:
        import antenv.axon_hooks  # noqa: F401
        return
    except ImportError:
        pass
    import types
    hook = None
    try:
        if "/root/.axon_site" not in sys.path:
            sys.path.insert(0, "/root/.axon_site")
        from trn_agent_boot.trn_boot import _ntff_profile_via_ctypes
        hook = _ntff_profile_via_ctypes("/opt/axon/libaxon_pjrt.so")
    except Exception:
        hook = None
    m = types.ModuleType("antenv.axon_hooks")
    m.get_axon_ntff_profile_hook = lambda: hook
    sys.modules["antenv.axon_hooks"] = m


def kernel(tokens, norm_w, w_q, w_kv, w_out, q_gamma, k_gamma):
    tokens = np.asarray(tokens, np.float32)
    norm_w = np.asarray(norm_w, np.float32)
    w_q = np.asarray(w_q, np.float32)
    w_kv = np.asarray(w_kv, np.float32)
    w_out = np.asarray(w_out, np.float32)
    q_gamma = np.asarray(q_gamma, np.float32)
    k_gamma = np.asarray(k_gamma, np.float32)

    bf = ml_dtypes.bfloat16
    wq_f = norm_w[:, None] * w_q
    wkv_f = norm_w[:, None] * w_kv
    wk_f = wkv_f[:, :H * DH]
    wv_f = wkv_f[:, H * DH:]
    tok_bf = np.ascontiguousarray(tokens.reshape(T, DIM).astype(bf).T)
    # combined q*k gamma scale (incl. both sqrt(DH) factors), applied on q side
    g2_full = ((q_gamma + 1.0) * (k_gamma + 1.0) * float(DH)).reshape(H * DH)

    def _swz(w):
        # [DIM, CD] -> device lhsT layout [p=128, (c, m)] contiguous
        return np.ascontiguousarray(
            w.astype(bf).reshape(DCH, 128, CD).transpose(1, 0, 2)
            .reshape(128, DCH * CD))

    in_maps = []
    for c in range(NCORES):
        cols = slice(c * CD, (c + 1) * CD)
        g2c = g2_full[c * CD:(c + 1) * CD]
        wo_c = w_out[cols, :]
        # attT rows are [head B dims, head A dims]
        wo_r = np.concatenate([wo_c[64:128, :], wo_c[0:64, :]], axis=0)
        in_maps.append({
            "tok": tok_bf,
            "wq": _swz(wq_f[:, cols]),
            "wk": _swz(wk_f[:, cols]),
            "wv": _swz(wv_f[:, cols]),
            "wo": np.ascontiguousarray(wo_r).astype(bf),
            "gq": np.ascontiguousarray(
                (1.0 / (g2c * g2c)).reshape(CD, 1), dtype=np.float32),
        })

    nc = _get_nc()
    trace = os.environ.get("KBENCH_TRACE") == "1"
    kwargs = {}
    if trace:
        _ensure_axon_hooks()
        import concourse.bass_utils as _bu
        _bu.upload_artifacts = lambda d: "local://" + d
        kwargs = {"trace": True,
                  "tmpdir": os.environ.get("KBENCH_TRACE_DIR") or None}
    res = run_bass_kernel_spmd(nc, in_maps, core_ids=list(range(NCORES)),
                               **kwargs)
    if res.exec_time_ns is not None:
        print(f"HW exec time: {res.exec_time_ns} ns")
    acc = np.zeros((T, DIM), np.float32)
    for i in range(NCORES):
        acc += res.results[i]["out"].astype(np.float32)
    return acc.reshape(B, N, DIM)


if __name__ == "__main__":
    rng = np.random.default_rng(0)
    inputs = {
        "tokens": rng.standard_normal((B, N, DIM), dtype=np.float32),
        "norm_w": np.ones((DIM,), np.float32),
        "w_q": rng.standard_normal((DIM, H * DH), dtype=np.float32) * 0.02,
        "w_kv": rng.standard_normal((DIM, 2 * H * DH), dtype=np.float32) * 0.02,
        "w_out": rng.standard_normal((H * DH, DIM), dtype=np.float32) * 0.02,
        "q_gamma": np.zeros((H, DH), np.float32),
        "k_gamma": np.zeros((H, DH), np.float32),
    }
    out = kernel(**inputs)
    print("out", out.shape, out.dtype, float(np.abs(out).max()))
